# revision 10
# baseline (speedup 1.0000x reference)
"""Masked FFN kernel for trn2 (8 NeuronCores, SPMD data-parallel over rows).

Math: out = (gelu(x @ W1 + b1) @ W2 + b2) * mask  with masked-out rows exactly 0.

Strategy:
  - Host compacts the (B*T) rows down to the ~50% active ones (mask != 0),
    shards them evenly across 8 cores, pads per-core row count to rc.
  - Device computes the FFN on compacted rows only, in transposed layout
    and bf16 operands (PE runs bf16 at 1 col/cycle and LDWEIGHTS hides
    fully under the moving stream, unlike fp32r's two-pass load):
      mm1: H^T[f, r] = sum_d W1[d, f] * X^T[d, r]   (W1 tile stationary)
      gelu+b1 fused on ScalarE (PSUM -> SBUF, bf16 out)
      mm2: Y^T[o, r] = sum_f W2[f, o] * H^T[f, r]   (W2 tile stationary)
      +b2 fused on ScalarE (PSUM -> SBUF bf16), DMA out.
  - Phase A iterates block-major (the big 512-row block first) so the PE
    has ~50us of runway on 1MB of x while the rest of x / W1 / W2 stream
    in; W1 lives in 8 resident 4-f_t groups and x in one tile per block
    to minimize first-touch semaphore stalls on the PE queue.
  - Host scatters Y rows back into a zero output.
"""

import numpy as np
import ml_dtypes

import concourse.tile as tile
from concourse import bacc, mybir
from concourse import bass_utils

N_CORES = 8
D = 1024      # model dim
F = 4096      # ffn dim
DT = D // 128  # 8 d-tiles
FT = F // 128  # 32 f-tiles
OT = D // 128  # 8 output tiles
FG = 4         # f-tiles per W1 slab group
G1 = FT // FG  # 8 W1 groups

F32 = mybir.dt.float32
BF16 = mybir.dt.bfloat16
NP_BF16 = ml_dtypes.bfloat16

_CACHE: dict = {}
LAST_RESULTS = None  # BassKernelResults of the most recent device run (for test harness)


def _ensure_trace_support():
    """If BASS_TRACE is set but the NTFF hook module is missing, install a
    local shim so run_bass_kernel_spmd's trace path works instead of crashing."""
    import os
    if not os.environ.get("BASS_TRACE"):
        return
    import sys, types
    try:
        import antenv.axon_hooks  # noqa: F401
    except ImportError:
        mod = types.ModuleType("antenv.axon_hooks")
        mod._h = None
        mod.set_axon_ntff_profile_hook = lambda h: setattr(mod, "_h", h)
        mod.get_axon_ntff_profile_hook = lambda: mod._h
        sys.modules["antenv.axon_hooks"] = mod
        try:
            from trn_agent_boot.trn_boot import _ntff_profile_via_ctypes
            mod.set_axon_ntff_profile_hook(
                _ntff_profile_via_ctypes("/opt/axon/libaxon_pjrt.so")
            )
        except Exception:
            pass
    try:
        bass_utils.upload_artifacts  # noqa: B018
        bass_utils.upload_artifacts = lambda tmpdir: tmpdir
    except Exception:
        pass


def _blocks(rc: int):
    """Split rc into moving-dim blocks of size in [256, 512] (rc < 256 -> [rc]),
    largest first."""
    if rc <= 512:
        return [(0, rc)]
    sizes = []
    rem = rc
    while rem > 512:
        b = min(512, rem - 256)
        sizes.append(b)
        rem -= b
    sizes.append(rem)
    out = []
    pos = 0
    for b in sizes:
        out.append((pos, b))
        pos += b
    return out


def _build(rc: int, nch: int):
    key = (rc, nch)
    if key in _CACHE:
        return _CACHE[key]

    nc = bacc.Bacc("TRN2", target_bir_lowering=False, debug=False, num_devices=N_CORES,
                   dynamic_dma_scratch_size=8192)
    xt_d = nc.dram_tensor("xt", [nch, 128, DT, rc], BF16, kind="ExternalInput").ap()
    w1_d = nc.dram_tensor("w1t", [G1, 128, FG, DT, 128], BF16, kind="ExternalInput").ap()
    b1_d = nc.dram_tensor("b1m", [128, FT], F32, kind="ExternalInput").ap()
    w2_d = nc.dram_tensor("w2t", [OT, 128, FT, 128], BF16, kind="ExternalInput").ap()
    b2_d = nc.dram_tensor("b2m", [128, OT], F32, kind="ExternalInput").ap()
    yt_d = nc.dram_tensor("yt", [nch, OT, 128, rc], BF16, kind="ExternalOutput").ap()

    blocks = _blocks(rc)
    gelu = mybir.ActivationFunctionType.Gelu_apprx_tanh
    ident = mybir.ActivationFunctionType.Identity
    W2_PREF = 2   # w2 panel prefetch distance (in o_t units)

    with tile.TileContext(nc) as tc:
        with (
            tc.tile_pool(name="consts", bufs=1) as consts,
            tc.tile_pool(name="xpool", bufs=len(blocks)) as xpool,
            tc.tile_pool(name="hpool", bufs=1) as hpool,
            tc.tile_pool(name="w1pool", bufs=G1) as w1pool,
            tc.tile_pool(name="w2pool", bufs=W2_PREF + 1) as w2pool,
            tc.tile_pool(name="ypool", bufs=3) as ypool,
            tc.tile_pool(name="pspool", bufs=8, space="PSUM") as pspool,
        ):
            b1_sb = consts.tile([128, FT], F32)
            b2_sb = consts.tile([128, OT], F32)

            def dma_w1(g, queue):
                w = w1pool.tile([128, FG, DT, 128], BF16, tag="w1")
                queue.dma_start(out=w, in_=w1_d[g])
                return w

            def dma_w2(o_t, queue):
                w = w2pool.tile([128, FT, 128], BF16, tag="w2")
                queue.dma_start(out=w, in_=w2_d[o_t])
                return w

            for ch in range(nch):
                # Tiles for the startup-critical data: x block 0 and W1
                # groups 0/1 are uploaded as per-slice DMAs interleaved
                # across both HWDGE queues in the order the PE consumes
                # them (multiple in-flight DMAs engage parallel engines).
                xt_sb = [
                    xpool.tile([128, DT, blk], BF16, tag="xt", name=f"xb{bi}")
                    for bi, (b0, blk) in enumerate(blocks)
                ]
                w1_sb = {
                    g: w1pool.tile([128, FG, DT, 128], BF16, tag="w1", name=f"w1g{g}")
                    for g in range(2)
                }
                b00, blk0 = blocks[0]

                def x0_dma(queue, d_t):
                    queue.dma_start(
                        out=xt_sb[0][:, d_t],
                        in_=xt_d[ch, :, d_t, b00 : b00 + blk0],
                    )

                def w1_slice_dma(queue, g, j):
                    queue.dma_start(out=w1_sb[g][:, j], in_=w1_d[g, :, j])

                w1_slice_dma(nc.scalar, 0, 0)
                x0_dma(nc.sync, 0)
                x0_dma(nc.scalar, 1)
                x0_dma(nc.sync, 2)
                x0_dma(nc.scalar, 3)
                x0_dma(nc.sync, 4)
                x0_dma(nc.scalar, 5)
                x0_dma(nc.sync, 6)
                x0_dma(nc.scalar, 7)
                w1_slice_dma(nc.sync, 0, 1)
                w1_slice_dma(nc.scalar, 0, 2)
                w1_slice_dma(nc.sync, 0, 3)
                w1_slice_dma(nc.scalar, 1, 0)
                w1_slice_dma(nc.sync, 1, 1)
                w1_slice_dma(nc.scalar, 1, 2)
                w1_slice_dma(nc.sync, 1, 3)
                if ch == 0:
                    nc.scalar.dma_start(out=b1_sb, in_=b1_d)
                    nc.scalar.dma_start(out=b2_sb, in_=b2_d)
                # remaining W1 groups and x blocks; W2 panel 0 for phase B
                for g in range(2, G1):
                    w1_sb[g] = dma_w1(g, nc.sync)
                w2_sb = {0: dma_w2(0, nc.scalar)}
                for bi, (b0, blk) in enumerate(blocks):
                    if bi == 0:
                        continue
                    nc.sync.dma_start(
                        out=xt_sb[bi], in_=xt_d[ch, :, :, b0 : b0 + blk]
                    )
                ht_sb = hpool.tile([128, FT, rc], BF16, tag="ht")

                # ---- phase A: H^T = gelu(W1^T-tiles @ X^T + b1), block-major ----
                for bi, (b0, blk) in enumerate(blocks):
                    for f_t in range(FT):
                        g, j = divmod(f_t, FG)
                        w1sb = w1_sb[g]
                        ps = pspool.tile([128, 512], F32, tag="ps")
                        for d_t in range(DT):
                            nc.tensor.matmul(
                                ps[:, :blk],
                                lhsT=w1sb[:, j, d_t, :],
                                rhs=xt_sb[bi][:, d_t, :],
                                start=(d_t == 0),
                                stop=(d_t == DT - 1),
                            )
                        nc.scalar.activation(
                            out=ht_sb[:, f_t, b0 : b0 + blk],
                            in_=ps[:, :blk],
                            func=gelu,
                            bias=b1_sb[:, f_t : f_t + 1],
                            scale=1.0,
                        )

                # ---- phase B: Y^T = W2^T-tiles @ H^T + b2 ----
                for i in range(1, min(W2_PREF, OT)):
                    w2_sb[i] = dma_w2(i, nc.sync)
                for o_t in range(OT):
                    po = o_t + W2_PREF
                    if po < OT and po not in w2_sb:
                        w2_sb[po] = dma_w2(po, nc.sync)
                    w2sb = w2_sb.pop(o_t)
                    for b0, blk in blocks:
                        ps2 = pspool.tile([128, 512], F32, tag="ps")
                        for f_t in range(FT):
                            nc.tensor.matmul(
                                ps2[:, :blk],
                                lhsT=w2sb[:, f_t, :],
                                rhs=ht_sb[:, f_t, b0 : b0 + blk],
                                start=(f_t == 0),
                                stop=(f_t == FT - 1),
                            )
                        yt_t = ypool.tile([128, 512], BF16, tag="yt")
                        nc.scalar.activation(
                            out=yt_t[:, :blk],
                            in_=ps2[:, :blk],
                            func=ident,
                            bias=b2_sb[:, o_t : o_t + 1],
                            scale=1.0,
                        )
                        nc.scalar.dma_start(
                            out=yt_d[ch, o_t, :, b0 : b0 + blk], in_=yt_t[:, :blk]
                        )

    nc.compile()
    _CACHE[key] = nc
    return nc


def _pick_shape(r_need: int):
    """Choose (rc, nch) given required per-core rows."""
    rc_max = 1100
    nch = 1
    while True:
        rc = -(-r_need // nch)  # ceil
        rc = max(256, rc)
        if rc <= rc_max:
            return rc, nch
        nch += 1


def kernel(inputs: np.ndarray, mask: np.ndarray, W1: np.ndarray, b1: np.ndarray,
           W2: np.ndarray, b2: np.ndarray) -> np.ndarray:
    global LAST_RESULTS
    B, T, Dm = inputs.shape
    assert Dm == D and W1.shape == (D, F) and W2.shape == (F, D)
    N = B * T

    x_flat = np.ascontiguousarray(np.asarray(inputs, dtype=np.float32).reshape(N, D))
    m_flat = np.asarray(mask).reshape(N).astype(bool)
    idx = np.flatnonzero(m_flat)
    na = idx.size
    out = np.zeros((N, D), dtype=np.float32)
    if na == 0:
        return out.reshape(B, T, D)

    r_need = -(-na // N_CORES)
    rc, nch = _pick_shape(r_need)
    cap = rc * nch

    nc = None
    while nc is None:
        try:
            nc = _build(rc, nch)
        except AssertionError:
            if nch >= 16:
                raise
            # SBUF overflow at this rc -> split into more chunks
            nch += 1
            rc = max(256, -(-r_need // nch))
            cap = rc * nch

    idx_pad = np.zeros(N_CORES * cap, dtype=np.int64)
    idx_pad[:na] = idx
    xg = x_flat[idx_pad].astype(NP_BF16)  # [N_CORES*cap, D]

    # weight/bias tilings (shared by all cores)
    # w1t[f_t, d_in, d_t, f] = W1[d_t*128+d_in, f_t*128+f], grouped by 4 f_t:
    # w1g[g, d_in, j, d_t, f] with f_t = 4g + j
    w1t = np.ascontiguousarray(
        np.asarray(W1, np.float32).reshape(DT, 128, FT, 128).transpose(2, 1, 0, 3)
    ).astype(NP_BF16)
    w1g = np.ascontiguousarray(
        w1t.reshape(G1, FG, 128, DT, 128).transpose(0, 2, 1, 3, 4)
    )
    # w2t[o_t, f_in, f_t, o] = W2[f_t*128+f_in, o_t*128+o]
    w2t = np.ascontiguousarray(
        np.asarray(W2, np.float32).reshape(FT, 128, OT, 128).transpose(2, 1, 0, 3)
    ).astype(NP_BF16)
    b1m = np.ascontiguousarray(np.asarray(b1, np.float32).reshape(FT, 128).T)
    b2m = np.ascontiguousarray(np.asarray(b2, np.float32).reshape(OT, 128).T)

    in_maps = []
    for c in range(N_CORES):
        xc = xg[c * cap : (c + 1) * cap]  # [cap, D] bf16
        xt = np.empty((nch, 128, DT, rc), dtype=NP_BF16)
        for ch in range(nch):
            # [d_in, d_t, r] layout: partition dim outermost
            xt[ch] = (
                xc[ch * rc : (ch + 1) * rc].T.reshape(DT, 128, rc).transpose(1, 0, 2)
            )
        in_maps.append({"xt": xt, "w1t": w1g, "b1m": b1m, "w2t": w2t, "b2m": b2m})

    _ensure_trace_support()
    res = bass_utils.run_bass_kernel_spmd(nc, in_maps, core_ids=list(range(N_CORES)))
    LAST_RESULTS = res

    y_parts = []
    for c in range(N_CORES):
        yt = res.results[c]["yt"]  # [nch, OT, 128, rc] bf16
        for ch in range(nch):
            y_parts.append(np.asarray(yt[ch], dtype=np.float32).reshape(D, rc).T)
    ycat = np.concatenate(y_parts, axis=0)  # [N_CORES*cap, D]
    out[idx] = ycat[:na]
    return out.reshape(B, T, D)


# revision 17
# speedup vs baseline: 1.1724x; 1.1724x over previous
"""Masked FFN kernel for trn2 (8 NeuronCores, SPMD data-parallel over rows).

Math: out = (gelu(x @ W1 + b1) @ W2 + b2) * mask  with masked-out rows exactly 0.

Strategy:
  - Host compacts the (B*T) rows down to the ~50% active ones (mask != 0),
    shards them evenly across 8 cores, pads per-core row count to rc.
  - Device computes the FFN on compacted rows only, in transposed layout
    and bf16 operands (PE runs bf16 at 1 col/cycle and LDWEIGHTS hides
    fully under the moving stream, unlike fp32r's two-pass load):
      mm1: H^T[f, r] = sum_d W1[d, f] * X^T[d, r]   (W1 tile stationary)
      gelu+b1 fused on ScalarE (PSUM -> SBUF, bf16 out)
      mm2: Y^T[o, r] = sum_f W2[f, o] * H^T[f, r]   (W2 tile stationary)
      +b2 fused on ScalarE (PSUM -> SBUF bf16), DMA out.
  - Phase A iterates block-major (the big 512-row block first) so the PE
    has ~50us of runway on 1MB of x while the rest of x / W1 / W2 stream
    in; W1 lives in 8 resident 4-f_t groups and x in one tile per block
    to minimize first-touch semaphore stalls on the PE queue.
  - Host scatters Y rows back into a zero output.
"""

import numpy as np
import ml_dtypes

import concourse.tile as tile
from concourse import bacc, mybir
from concourse import bass_utils

N_CORES = 8
D = 1024      # model dim
F = 4096      # ffn dim
DT = D // 128  # 8 d-tiles
FT = F // 128  # 32 f-tiles
OT = D // 128  # 8 output tiles
FG = 4         # f-tiles per W1 slab group
G1 = FT // FG  # 8 W1 groups

F32 = mybir.dt.float32
BF16 = mybir.dt.bfloat16
NP_BF16 = ml_dtypes.bfloat16

_CACHE: dict = {}
LAST_RESULTS = None  # BassKernelResults of the most recent device run (for test harness)


def _ensure_trace_support():
    """If BASS_TRACE is set but the NTFF hook module is missing, install a
    local shim so run_bass_kernel_spmd's trace path works instead of crashing."""
    import os
    if not os.environ.get("BASS_TRACE"):
        return
    import sys, types
    try:
        import antenv.axon_hooks  # noqa: F401
    except ImportError:
        mod = types.ModuleType("antenv.axon_hooks")
        mod._h = None
        mod.set_axon_ntff_profile_hook = lambda h: setattr(mod, "_h", h)
        mod.get_axon_ntff_profile_hook = lambda: mod._h
        sys.modules["antenv.axon_hooks"] = mod
        try:
            from trn_agent_boot.trn_boot import _ntff_profile_via_ctypes
            mod.set_axon_ntff_profile_hook(
                _ntff_profile_via_ctypes("/opt/axon/libaxon_pjrt.so")
            )
        except Exception:
            pass
    try:
        bass_utils.upload_artifacts  # noqa: B018
        bass_utils.upload_artifacts = lambda tmpdir: tmpdir
    except Exception:
        pass


def _blocks(rc: int):
    """Split rc into moving-dim blocks of size in [256, 512] (rc < 256 -> [rc]),
    largest first."""
    if rc <= 512:
        return [(0, rc)]
    sizes = []
    rem = rc
    while rem > 512:
        b = min(512, rem - 256)
        sizes.append(b)
        rem -= b
    sizes.append(rem)
    out = []
    pos = 0
    for b in sizes:
        out.append((pos, b))
        pos += b
    return out


def _build(rc: int, nch: int):
    key = (rc, nch)
    if key in _CACHE:
        return _CACHE[key]

    nc = bacc.Bacc("TRN2", target_bir_lowering=False, debug=False, num_devices=N_CORES,
                   dynamic_dma_scratch_size=8192)
    xt_d = nc.dram_tensor("xt", [nch, 128, DT, rc], BF16, kind="ExternalInput").ap()
    w1_d = nc.dram_tensor("w1t", [G1, 128, FG, DT, 128], BF16, kind="ExternalInput").ap()
    b1_d = nc.dram_tensor("b1m", [128, FT], F32, kind="ExternalInput").ap()
    w2_d = nc.dram_tensor("w2t", [OT // 2, 128, 2, FT, 128], BF16, kind="ExternalInput").ap()
    b2_d = nc.dram_tensor("b2m", [128, OT], F32, kind="ExternalInput").ap()
    yt_d = nc.dram_tensor("yt", [nch, OT, 128, rc], BF16, kind="ExternalOutput").ap()

    blocks = _blocks(rc)
    gelu = mybir.ActivationFunctionType.Gelu_apprx_tanh
    ident = mybir.ActivationFunctionType.Identity
    W2_PREF = 2   # w2 panel prefetch distance (in o_t units)

    with tile.TileContext(nc) as tc:
        with (
            tc.tile_pool(name="consts", bufs=1) as consts,
            tc.tile_pool(name="xpool", bufs=1) as xpool,
            tc.tile_pool(name="hpool", bufs=1) as hpool,
            tc.tile_pool(name="w1pool", bufs=G1) as w1pool,
            tc.tile_pool(name="w2pool", bufs=W2_PREF + 1) as w2pool,
            tc.tile_pool(name="ypool", bufs=3) as ypool,
            tc.tile_pool(name="pspool", bufs=8, space="PSUM") as pspool,
        ):
            b1_sb = consts.tile([128, FT], F32)
            b2_sb = consts.tile([128, OT], F32)

            def dma_w1(g, queue):
                w = w1pool.tile([128, FG, DT, 128], BF16, tag="w1")
                queue.dma_start(out=w, in_=w1_d[g])
                return w

            def dma_w2(dp, queue):
                w = w2pool.tile([128, 2, FT, 128], BF16, tag="w2")
                queue.dma_start(out=w, in_=w2_d[dp])
                return w

            for ch in range(nch):
                # Tiles for the startup-critical data: x block 0 and W1
                # groups 0/1 are uploaded as per-slice DMAs interleaved
                # across both HWDGE queues in the order the PE consumes
                # them (multiple in-flight DMAs engage parallel engines).
                b00, blk0 = blocks[0]
                rrest = rc - blk0
                x0_sb = xpool.tile([128, DT, blk0], BF16, tag="x0", name="xb0")
                xr_sb = (
                    xpool.tile([128, DT, rrest], BF16, tag="xr", name="xbr")
                    if rrest > 0
                    else None
                )
                w1_sb = {
                    g: w1pool.tile([128, FG, DT, 128], BF16, tag="w1", name=f"w1g{g}")
                    for g in range(2)
                }

                def x0_dma(queue, d_t):
                    queue.dma_start(
                        out=x0_sb[:, d_t],
                        in_=xt_d[ch, :, d_t, b00 : b00 + blk0],
                    )

                def w1_slice_dma(queue, g, j):
                    queue.dma_start(out=w1_sb[g][:, j], in_=w1_d[g, :, j])

                # g0 j0 split in halves across both queues so the f_t=0
                # chain's stationary tiles land in ~1us
                nc.scalar.dma_start(out=w1_sb[0][:, 0, 0:4], in_=w1_d[0, :, 0, 0:4])
                x0_dma(nc.sync, 0)
                nc.sync.dma_start(out=w1_sb[0][:, 0, 4:8], in_=w1_d[0, :, 0, 4:8])
                x0_dma(nc.scalar, 1)
                x0_dma(nc.sync, 2)
                x0_dma(nc.scalar, 3)
                x0_dma(nc.sync, 4)
                x0_dma(nc.scalar, 5)
                x0_dma(nc.sync, 6)
                x0_dma(nc.scalar, 7)
                w1_slice_dma(nc.sync, 0, 1)
                w1_slice_dma(nc.scalar, 0, 2)
                w1_slice_dma(nc.sync, 0, 3)
                w1_slice_dma(nc.scalar, 1, 0)
                w1_slice_dma(nc.sync, 1, 1)
                w1_slice_dma(nc.scalar, 1, 2)
                w1_slice_dma(nc.sync, 1, 3)
                if ch == 0:
                    nc.scalar.dma_start(out=b1_sb, in_=b1_d)
                    nc.scalar.dma_start(out=b2_sb, in_=b2_d)
                # remaining W1 groups, rest of x (one DMA), first W2 double-panel
                for g in range(2, G1):
                    w1_sb[g] = dma_w1(g, nc.sync)
                w2_sb = {0: dma_w2(0, nc.scalar)}
                if xr_sb is not None:
                    nc.sync.dma_start(out=xr_sb, in_=xt_d[ch, :, :, blk0:rc])
                ht_sb = hpool.tile([128, FT, rc], BF16, tag="ht")

                def x_rhs(bi, d_t):
                    if bi == 0:
                        return x0_sb[:, d_t, :]
                    b0, blk = blocks[bi]
                    off = b0 - blk0
                    return xr_sb[:, d_t, off : off + blk]

                # ---- phase A: H^T = gelu(W1^T-tiles @ X^T + b1), block-major ----
                for bi, (b0, blk) in enumerate(blocks):
                    for f_t in range(FT):
                        g, j = divmod(f_t, FG)
                        w1sb = w1_sb[g]
                        ps = pspool.tile([128, 512], F32, tag="ps")
                        for d_t in range(DT):
                            nc.tensor.matmul(
                                ps[:, :blk],
                                lhsT=w1sb[:, j, d_t, :],
                                rhs=x_rhs(bi, d_t),
                                start=(d_t == 0),
                                stop=(d_t == DT - 1),
                            )
                        nc.scalar.activation(
                            out=ht_sb[:, f_t, b0 : b0 + blk],
                            in_=ps[:, :blk],
                            func=gelu,
                            bias=b1_sb[:, f_t : f_t + 1],
                            scale=1.0,
                        )

                # ---- phase B: Y^T = W2^T-tiles @ H^T + b2 ----
                # W2 rides in 2-o_t double-panels: half the bulk-DMA
                # completion events (each costs the PE ~430ns).
                NDP = OT // 2
                for o_t in range(OT):
                    dp, half = divmod(o_t, 2)
                    if half == 0:
                        pd = dp + 1
                        if pd < NDP and pd not in w2_sb:
                            w2_sb[pd] = dma_w2(pd, nc.sync)
                    w2sb = w2_sb[dp] if half == 0 else w2_sb.pop(dp)
                    for b0, blk in blocks:
                        ps2 = pspool.tile([128, 512], F32, tag="ps")
                        for f_t in range(FT):
                            nc.tensor.matmul(
                                ps2[:, :blk],
                                lhsT=w2sb[:, half, f_t, :],
                                rhs=ht_sb[:, f_t, b0 : b0 + blk],
                                start=(f_t == 0),
                                stop=(f_t == FT - 1),
                            )
                        yt_t = ypool.tile([128, 512], BF16, tag="yt")
                        nc.scalar.activation(
                            out=yt_t[:, :blk],
                            in_=ps2[:, :blk],
                            func=ident,
                            bias=b2_sb[:, o_t : o_t + 1],
                            scale=1.0,
                        )
                        nc.scalar.dma_start(
                            out=yt_d[ch, o_t, :, b0 : b0 + blk], in_=yt_t[:, :blk]
                        )

    nc.compile()
    _CACHE[key] = nc
    return nc


def _pick_shape(r_need: int):
    """Choose (rc, nch) given required per-core rows."""
    rc_max = 1100
    nch = 1
    while True:
        rc = -(-r_need // nch)  # ceil
        rc = max(256, rc)
        if rc <= rc_max:
            return rc, nch
        nch += 1


def kernel(inputs: np.ndarray, mask: np.ndarray, W1: np.ndarray, b1: np.ndarray,
           W2: np.ndarray, b2: np.ndarray) -> np.ndarray:
    global LAST_RESULTS
    B, T, Dm = inputs.shape
    assert Dm == D and W1.shape == (D, F) and W2.shape == (F, D)
    N = B * T

    x_flat = np.ascontiguousarray(np.asarray(inputs, dtype=np.float32).reshape(N, D))
    m_flat = np.asarray(mask).reshape(N).astype(bool)
    idx = np.flatnonzero(m_flat)
    na = idx.size
    out = np.zeros((N, D), dtype=np.float32)
    if na == 0:
        return out.reshape(B, T, D)

    r_need = -(-na // N_CORES)
    rc, nch = _pick_shape(r_need)
    cap = rc * nch

    nc = None
    while nc is None:
        try:
            nc = _build(rc, nch)
        except AssertionError:
            if nch >= 16:
                raise
            # SBUF overflow at this rc -> split into more chunks
            nch += 1
            rc = max(256, -(-r_need // nch))
            cap = rc * nch

    idx_pad = np.zeros(N_CORES * cap, dtype=np.int64)
    idx_pad[:na] = idx
    xg = x_flat[idx_pad].astype(NP_BF16)  # [N_CORES*cap, D]

    # weight/bias tilings (shared by all cores)
    # w1t[f_t, d_in, d_t, f] = W1[d_t*128+d_in, f_t*128+f], grouped by 4 f_t:
    # w1g[g, d_in, j, d_t, f] with f_t = 4g + j
    w1t = np.ascontiguousarray(
        np.asarray(W1, np.float32).reshape(DT, 128, FT, 128).transpose(2, 1, 0, 3)
    ).astype(NP_BF16)
    w1g = np.ascontiguousarray(
        w1t.reshape(G1, FG, 128, DT, 128).transpose(0, 2, 1, 3, 4)
    )
    # w2t[o_t, f_in, f_t, o] = W2[f_t*128+f_in, o_t*128+o], paired into
    # double-panels: w2g[dp, f_in, half, f_t, o] with o_t = 2*dp + half
    w2t = np.ascontiguousarray(
        np.asarray(W2, np.float32).reshape(FT, 128, OT, 128).transpose(2, 1, 0, 3)
    ).astype(NP_BF16)
    w2g = np.ascontiguousarray(
        w2t.reshape(OT // 2, 2, 128, FT, 128).transpose(0, 2, 1, 3, 4)
    )
    b1m = np.ascontiguousarray(np.asarray(b1, np.float32).reshape(FT, 128).T)
    b2m = np.ascontiguousarray(np.asarray(b2, np.float32).reshape(OT, 128).T)

    in_maps = []
    for c in range(N_CORES):
        xc = xg[c * cap : (c + 1) * cap]  # [cap, D] bf16
        xt = np.empty((nch, 128, DT, rc), dtype=NP_BF16)
        for ch in range(nch):
            # [d_in, d_t, r] layout: partition dim outermost
            xt[ch] = (
                xc[ch * rc : (ch + 1) * rc].T.reshape(DT, 128, rc).transpose(1, 0, 2)
            )
        in_maps.append({"xt": xt, "w1t": w1g, "b1m": b1m, "w2t": w2g, "b2m": b2m})

    _ensure_trace_support()
    res = bass_utils.run_bass_kernel_spmd(nc, in_maps, core_ids=list(range(N_CORES)))
    LAST_RESULTS = res

    y_parts = []
    for c in range(N_CORES):
        yt = res.results[c]["yt"]  # [nch, OT, 128, rc] bf16
        for ch in range(nch):
            y_parts.append(np.asarray(yt[ch], dtype=np.float32).reshape(D, rc).T)
    ycat = np.concatenate(y_parts, axis=0)  # [N_CORES*cap, D]
    out[idx] = ycat[:na]
    return out.reshape(B, T, D)


# revision 18
# speedup vs baseline: 1.1900x; 1.0149x over previous
"""Masked FFN kernel for trn2 (8 NeuronCores, SPMD data-parallel over rows).

Math: out = (gelu(x @ W1 + b1) @ W2 + b2) * mask  with masked-out rows exactly 0.

Strategy:
  - Host compacts the (B*T) rows down to the ~50% active ones (mask != 0),
    shards them evenly across 8 cores, pads per-core row count to rc.
  - Device computes the FFN on compacted rows only, in transposed layout
    and bf16 operands (PE runs bf16 at 1 col/cycle and LDWEIGHTS hides
    fully under the moving stream, unlike fp32r's two-pass load):
      mm1: H^T[f, r] = sum_d W1[d, f] * X^T[d, r]   (W1 tile stationary)
      gelu+b1 fused on ScalarE (PSUM -> SBUF, bf16 out)
      mm2: Y^T[o, r] = sum_f W2[f, o] * H^T[f, r]   (W2 tile stationary)
      +b2 fused on ScalarE (PSUM -> SBUF bf16), DMA out.
  - Phase A iterates block-major (the big 512-row block first) so the PE
    has ~50us of runway on 1MB of x while the rest of x / W1 / W2 stream
    in; W1 lives in 8 resident 4-f_t groups and x in one tile per block
    to minimize first-touch semaphore stalls on the PE queue.
  - Host scatters Y rows back into a zero output.
"""

import numpy as np
import ml_dtypes

import concourse.tile as tile
from concourse import bacc, mybir
from concourse import bass_utils

N_CORES = 8
D = 1024      # model dim
F = 4096      # ffn dim
DT = D // 128  # 8 d-tiles
FT = F // 128  # 32 f-tiles
OT = D // 128  # 8 output tiles
FG = 4         # f-tiles per W1 slab group
G1 = FT // FG  # 8 W1 groups

F32 = mybir.dt.float32
BF16 = mybir.dt.bfloat16
NP_BF16 = ml_dtypes.bfloat16

_CACHE: dict = {}
LAST_RESULTS = None  # BassKernelResults of the most recent device run (for test harness)


def _ensure_trace_support():
    """If BASS_TRACE is set but the NTFF hook module is missing, install a
    local shim so run_bass_kernel_spmd's trace path works instead of crashing."""
    import os
    if not os.environ.get("BASS_TRACE"):
        return
    import sys, types
    try:
        import antenv.axon_hooks  # noqa: F401
    except ImportError:
        mod = types.ModuleType("antenv.axon_hooks")
        mod._h = None
        mod.set_axon_ntff_profile_hook = lambda h: setattr(mod, "_h", h)
        mod.get_axon_ntff_profile_hook = lambda: mod._h
        sys.modules["antenv.axon_hooks"] = mod
        try:
            from trn_agent_boot.trn_boot import _ntff_profile_via_ctypes
            mod.set_axon_ntff_profile_hook(
                _ntff_profile_via_ctypes("/opt/axon/libaxon_pjrt.so")
            )
        except Exception:
            pass
    try:
        bass_utils.upload_artifacts  # noqa: B018
        bass_utils.upload_artifacts = lambda tmpdir: tmpdir
    except Exception:
        pass


def _blocks(rc: int):
    """Split rc into moving-dim blocks of size in [256, 512] (rc < 256 -> [rc]),
    largest first."""
    if rc <= 512:
        return [(0, rc)]
    sizes = []
    rem = rc
    while rem > 512:
        b = min(512, rem - 256)
        sizes.append(b)
        rem -= b
    sizes.append(rem)
    out = []
    pos = 0
    for b in sizes:
        out.append((pos, b))
        pos += b
    return out


def _build(rc: int, nch: int):
    key = (rc, nch)
    if key in _CACHE:
        return _CACHE[key]

    nc = bacc.Bacc("TRN2", target_bir_lowering=False, debug=False, num_devices=N_CORES,
                   dynamic_dma_scratch_size=8192)
    xt_d = nc.dram_tensor("xt", [nch, 128, DT, rc], BF16, kind="ExternalInput").ap()
    w1_d = nc.dram_tensor("w1t", [G1, 128, FG, DT, 128], BF16, kind="ExternalInput").ap()
    b1_d = nc.dram_tensor("b1m", [128, FT], F32, kind="ExternalInput").ap()
    w2_d = nc.dram_tensor("w2t", [OT // 2, 128, 2, FT, 128], BF16, kind="ExternalInput").ap()
    b2_d = nc.dram_tensor("b2m", [128, OT], F32, kind="ExternalInput").ap()
    yt_d = nc.dram_tensor("yt", [nch, OT, 128, rc], BF16, kind="ExternalOutput").ap()

    blocks = _blocks(rc)
    gelu = mybir.ActivationFunctionType.Gelu_apprx_tanh
    ident = mybir.ActivationFunctionType.Identity
    W2_PREF = 2   # w2 panel prefetch distance (in o_t units)

    with tile.TileContext(nc) as tc:
        with (
            tc.tile_pool(name="consts", bufs=1) as consts,
            tc.tile_pool(name="xpool", bufs=1) as xpool,
            tc.tile_pool(name="hpool", bufs=1) as hpool,
            tc.tile_pool(name="w1pool", bufs=G1) as w1pool,
            tc.tile_pool(name="w2pool", bufs=W2_PREF + 1) as w2pool,
            tc.tile_pool(name="ypool", bufs=3) as ypool,
            tc.tile_pool(name="pspool", bufs=8, space="PSUM") as pspool,
        ):
            b1_sb = consts.tile([128, FT], F32)
            b2_sb = consts.tile([128, OT], F32)

            def dma_w1(g, queue):
                w = w1pool.tile([128, FG, DT, 128], BF16, tag="w1")
                queue.dma_start(out=w, in_=w1_d[g])
                return w

            def dma_w2(dp, queue):
                w = w2pool.tile([128, 2, FT, 128], BF16, tag="w2")
                queue.dma_start(out=w, in_=w2_d[dp])
                return w

            for ch in range(nch):
                # Tiles for the startup-critical data: x block 0 and W1
                # groups 0/1 are uploaded as per-slice DMAs interleaved
                # across both HWDGE queues in the order the PE consumes
                # them (multiple in-flight DMAs engage parallel engines).
                b00, blk0 = blocks[0]
                rrest = rc - blk0
                x0_sb = xpool.tile([128, DT, blk0], BF16, tag="x0", name="xb0")
                xr_sb = (
                    xpool.tile([128, DT, rrest], BF16, tag="xr", name="xbr")
                    if rrest > 0
                    else None
                )
                w1_sb = {
                    g: w1pool.tile([128, FG, DT, 128], BF16, tag="w1", name=f"w1g{g}")
                    for g in range(2)
                }

                def x0_dma(queue, d_t):
                    queue.dma_start(
                        out=x0_sb[:, d_t],
                        in_=xt_d[ch, :, d_t, b00 : b00 + blk0],
                    )

                def w1_slice_dma(queue, g, j):
                    queue.dma_start(out=w1_sb[g][:, j], in_=w1_d[g, :, j])

                # g0 j0 split in halves across both queues so the f_t=0
                # chain's stationary tiles land in ~1us; j1/j2 interleave
                # mid-stream (needed right as chain f0 ends, before x5/x7).
                nc.scalar.dma_start(out=w1_sb[0][:, 0, 0:4], in_=w1_d[0, :, 0, 0:4])
                x0_dma(nc.sync, 0)
                nc.sync.dma_start(out=w1_sb[0][:, 0, 4:8], in_=w1_d[0, :, 0, 4:8])
                x0_dma(nc.scalar, 1)
                x0_dma(nc.sync, 2)
                x0_dma(nc.scalar, 3)
                w1_slice_dma(nc.sync, 0, 1)
                w1_slice_dma(nc.scalar, 0, 2)
                x0_dma(nc.sync, 4)
                x0_dma(nc.scalar, 5)
                x0_dma(nc.sync, 6)
                x0_dma(nc.scalar, 7)
                w1_slice_dma(nc.sync, 0, 3)
                w1_slice_dma(nc.scalar, 1, 0)
                w1_slice_dma(nc.sync, 1, 1)
                w1_slice_dma(nc.scalar, 1, 2)
                w1_slice_dma(nc.sync, 1, 3)
                if ch == 0:
                    nc.scalar.dma_start(out=b1_sb, in_=b1_d)
                    nc.scalar.dma_start(out=b2_sb, in_=b2_d)
                # remaining W1 groups, rest of x (one DMA), first W2 double-panel
                for g in range(2, G1):
                    w1_sb[g] = dma_w1(g, nc.sync)
                w2_sb = {0: dma_w2(0, nc.scalar)}
                if xr_sb is not None:
                    nc.sync.dma_start(out=xr_sb, in_=xt_d[ch, :, :, blk0:rc])
                ht_sb = hpool.tile([128, FT, rc], BF16, tag="ht")

                def x_rhs(bi, d_t):
                    if bi == 0:
                        return x0_sb[:, d_t, :]
                    b0, blk = blocks[bi]
                    off = b0 - blk0
                    return xr_sb[:, d_t, off : off + blk]

                # ---- phase A: H^T = gelu(W1^T-tiles @ X^T + b1), block-major ----
                for bi, (b0, blk) in enumerate(blocks):
                    for f_t in range(FT):
                        g, j = divmod(f_t, FG)
                        w1sb = w1_sb[g]
                        ps = pspool.tile([128, 512], F32, tag="ps")
                        for d_t in range(DT):
                            nc.tensor.matmul(
                                ps[:, :blk],
                                lhsT=w1sb[:, j, d_t, :],
                                rhs=x_rhs(bi, d_t),
                                start=(d_t == 0),
                                stop=(d_t == DT - 1),
                            )
                        nc.scalar.activation(
                            out=ht_sb[:, f_t, b0 : b0 + blk],
                            in_=ps[:, :blk],
                            func=gelu,
                            bias=b1_sb[:, f_t : f_t + 1],
                            scale=1.0,
                        )

                # ---- phase B: Y^T = W2^T-tiles @ H^T + b2 ----
                # W2 rides in 2-o_t double-panels: half the bulk-DMA
                # completion events (each costs the PE ~430ns).
                NDP = OT // 2
                for o_t in range(OT):
                    dp, half = divmod(o_t, 2)
                    if half == 0:
                        pd = dp + 1
                        if pd < NDP and pd not in w2_sb:
                            w2_sb[pd] = dma_w2(pd, nc.sync)
                    w2sb = w2_sb[dp] if half == 0 else w2_sb.pop(dp)
                    for b0, blk in blocks:
                        ps2 = pspool.tile([128, 512], F32, tag="ps")
                        for f_t in range(FT):
                            nc.tensor.matmul(
                                ps2[:, :blk],
                                lhsT=w2sb[:, half, f_t, :],
                                rhs=ht_sb[:, f_t, b0 : b0 + blk],
                                start=(f_t == 0),
                                stop=(f_t == FT - 1),
                            )
                        yt_t = ypool.tile([128, 512], BF16, tag="yt")
                        nc.scalar.activation(
                            out=yt_t[:, :blk],
                            in_=ps2[:, :blk],
                            func=ident,
                            bias=b2_sb[:, o_t : o_t + 1],
                            scale=1.0,
                        )
                        nc.scalar.dma_start(
                            out=yt_d[ch, o_t, :, b0 : b0 + blk], in_=yt_t[:, :blk]
                        )

    nc.compile()
    _CACHE[key] = nc
    return nc


def _pick_shape(r_need: int):
    """Choose (rc, nch) given required per-core rows."""
    rc_max = 1100
    nch = 1
    while True:
        rc = -(-r_need // nch)  # ceil
        rc = max(256, rc)
        if rc <= rc_max:
            return rc, nch
        nch += 1


def kernel(inputs: np.ndarray, mask: np.ndarray, W1: np.ndarray, b1: np.ndarray,
           W2: np.ndarray, b2: np.ndarray) -> np.ndarray:
    global LAST_RESULTS
    B, T, Dm = inputs.shape
    assert Dm == D and W1.shape == (D, F) and W2.shape == (F, D)
    N = B * T

    x_flat = np.ascontiguousarray(np.asarray(inputs, dtype=np.float32).reshape(N, D))
    m_flat = np.asarray(mask).reshape(N).astype(bool)
    idx = np.flatnonzero(m_flat)
    na = idx.size
    out = np.zeros((N, D), dtype=np.float32)
    if na == 0:
        return out.reshape(B, T, D)

    r_need = -(-na // N_CORES)
    rc, nch = _pick_shape(r_need)
    cap = rc * nch

    nc = None
    while nc is None:
        try:
            nc = _build(rc, nch)
        except AssertionError:
            if nch >= 16:
                raise
            # SBUF overflow at this rc -> split into more chunks
            nch += 1
            rc = max(256, -(-r_need // nch))
            cap = rc * nch

    idx_pad = np.zeros(N_CORES * cap, dtype=np.int64)
    idx_pad[:na] = idx
    xg = x_flat[idx_pad].astype(NP_BF16)  # [N_CORES*cap, D]

    # weight/bias tilings (shared by all cores)
    # w1t[f_t, d_in, d_t, f] = W1[d_t*128+d_in, f_t*128+f], grouped by 4 f_t:
    # w1g[g, d_in, j, d_t, f] with f_t = 4g + j
    w1t = np.ascontiguousarray(
        np.asarray(W1, np.float32).reshape(DT, 128, FT, 128).transpose(2, 1, 0, 3)
    ).astype(NP_BF16)
    w1g = np.ascontiguousarray(
        w1t.reshape(G1, FG, 128, DT, 128).transpose(0, 2, 1, 3, 4)
    )
    # w2t[o_t, f_in, f_t, o] = W2[f_t*128+f_in, o_t*128+o], paired into
    # double-panels: w2g[dp, f_in, half, f_t, o] with o_t = 2*dp + half
    w2t = np.ascontiguousarray(
        np.asarray(W2, np.float32).reshape(FT, 128, OT, 128).transpose(2, 1, 0, 3)
    ).astype(NP_BF16)
    w2g = np.ascontiguousarray(
        w2t.reshape(OT // 2, 2, 128, FT, 128).transpose(0, 2, 1, 3, 4)
    )
    b1m = np.ascontiguousarray(np.asarray(b1, np.float32).reshape(FT, 128).T)
    b2m = np.ascontiguousarray(np.asarray(b2, np.float32).reshape(OT, 128).T)

    in_maps = []
    for c in range(N_CORES):
        xc = xg[c * cap : (c + 1) * cap]  # [cap, D] bf16
        xt = np.empty((nch, 128, DT, rc), dtype=NP_BF16)
        for ch in range(nch):
            # [d_in, d_t, r] layout: partition dim outermost
            xt[ch] = (
                xc[ch * rc : (ch + 1) * rc].T.reshape(DT, 128, rc).transpose(1, 0, 2)
            )
        in_maps.append({"xt": xt, "w1t": w1g, "b1m": b1m, "w2t": w2g, "b2m": b2m})

    _ensure_trace_support()
    res = bass_utils.run_bass_kernel_spmd(nc, in_maps, core_ids=list(range(N_CORES)))
    LAST_RESULTS = res

    y_parts = []
    for c in range(N_CORES):
        yt = res.results[c]["yt"]  # [nch, OT, 128, rc] bf16
        for ch in range(nch):
            y_parts.append(np.asarray(yt[ch], dtype=np.float32).reshape(D, rc).T)
    ycat = np.concatenate(y_parts, axis=0)  # [N_CORES*cap, D]
    out[idx] = ycat[:na]
    return out.reshape(B, T, D)
